# revision 15
# baseline (speedup 1.0000x reference)
"""Trainium2 Bass kernel for pointer-network greedy decode (sparse_attention).

Problem: B=256 batches, N=384 nodes, D=128, H*Hd=128. Sequential greedy
decode with visited masking, tanh-clipped bilinear scores.

Key algebraic reformulation: scores sum over all heads/dims, so
    raw[b,n] = ctx[b]^T (qv_flat kv_flat^T) hvec[b,n]
with ctx = hbar + h_last + h_first.  Precompute per-batch pairwise matrix
    A[b] = (hvec[b] @ qf_sc) @ (hvec[b] @ kf)^T     (qf_sc = 0.25*qf)
Then each decode step is:  x = base[b] + A[b][a_last,:] + A[b][a_first,:]
(a row gather + adds instead of matmuls).

Selection must reproduce XLA-CPU fp32 tanh tie semantics: tanh(x)==1.0 iff
x >= L_SAT (=7.9988117f). Selection rule (validated bit-exact vs reference
on the problem seed): first unvisited index with x >= T, where
    T = min(max_unvisited(x), L_SAT),  or -1e6 if max <= -L_SAT
Masking is additive (-1e9 into the persistent base tile).

Sharding: pure data-parallel over batch, 8 cores x 32 batches.
"""

import os
import numpy as np

import concourse.bass as bass
import concourse.bacc as bacc
import concourse.mybir as mybir
from concourse.bass import IndirectOffsetOnAxis
from concourse.bass_utils import run_bass_kernel_spmd
from concourse.masks import make_identity
from concourse.tile import TileContext

F32 = mybir.dt.float32
U32 = mybir.dt.uint32
I32 = mybir.dt.int32
U8 = mybir.dt.uint8

B_FULL = 256
N_CORES = 8
BL = B_FULL // N_CORES  # 32 batches per core
N = 384
D = 128

L_SAT = 7.9988117  # np.float32: smallest x with XLA-cpu tanh(x) == 1.0
NEG_BIG = -1.0e9
NEG_MED = -1.0e6
EXP_NEG10_SCALE = 10.0

AX_X = mybir.AxisListType.X
Alu = mybir.AluOpType
Act = mybir.ActivationFunctionType


def build_nc(n_steps: int = N, compile: bool = True) -> bass.Bass:
    nc = bacc.Bacc()

    hvec_in = nc.dram_tensor("hvec", [BL, N, D], F32, kind="ExternalInput")
    qf_in = nc.dram_tensor("qf", [D, D], F32, kind="ExternalInput")   # pre-scaled by 0.25
    kf_in = nc.dram_tensor("kf", [D, D], F32, kind="ExternalInput")
    # per-batch context projections, transposed: columns (2b, 2b+1) = (qhbar_sc[b], q0_sc[b])
    qh2_in = nc.dram_tensor("qh2", [D, 2 * BL], F32, kind="ExternalInput")

    pi_out = nc.dram_tensor("pi", [BL, N], I32, kind="ExternalOutput")
    logp_out = nc.dram_tensor("logp", [BL, 1], F32, kind="ExternalOutput")

    a_dram = nc.dram_tensor("a_mat", [BL * N, N], F32)       # pairwise rows
    bases_dram = nc.dram_tensor("bases", [BL, 2, N], F32)    # (base, x0) per b

    with TileContext(nc) as tc:
        with (
            tc.tile_pool(name="const", bufs=1) as cpool,
            tc.tile_pool(name="p1", bufs=2) as p1,
            tc.tile_pool(name="psum", bufs=2, space="PSUM") as psum,
            tc.tile_pool(name="loop", bufs=3) as lp,
            tc.tile_pool(name="pers", bufs=1) as pers,
        ):
            # ---------------- constants ----------------
            ident = cpool.tile([128, 128], F32, tag="ident")
            make_identity(nc, ident[:])
            qf_sb = cpool.tile([D, D], F32, tag="qf")
            nc.sync.dma_start(out=qf_sb[:], in_=qf_in[:])
            kf_sb = cpool.tile([D, D], F32, tag="kf")
            nc.sync.dma_start(out=kf_sb[:], in_=kf_in[:])
            qh2_sb = cpool.tile([D, 2 * BL], F32, tag="qh2")
            nc.sync.dma_start(out=qh2_sb[:], in_=qh2_in[:])

            # ---------------- phase 1: A = Qsc @ K^T per batch ----------------
            for b in range(BL):
                hv = p1.tile([128, 3, 128], F32, tag="hv")
                # hv[p, c, d] = hvec[b, c*128+p, d]
                nc.sync.dma_start(out=hv[:], in_=hvec_in[b].rearrange("(c p) d -> p c d", p=128))
                hvT = p1.tile([128, N], F32, tag="hvT")
                for c in range(3):
                    tps = psum.tile([128, 128], F32, tag="tps")
                    nc.tensor.transpose(out=tps[:], in_=hv[:, c, :], identity=ident[:])
                    nc.vector.tensor_copy(out=hvT[:, c * 128:(c + 1) * 128], in_=tps[:])
                qt_ps = psum.tile([128, N], F32, tag="qkps")
                nc.tensor.matmul(out=qt_ps[:], lhsT=qf_sb[:], rhs=hvT[:], start=True, stop=True)
                qt = p1.tile([128, N], F32, tag="qt")
                nc.vector.tensor_copy(out=qt[:], in_=qt_ps[:])
                kt_ps = psum.tile([128, N], F32, tag="qkps")
                nc.tensor.matmul(out=kt_ps[:], lhsT=kf_sb[:], rhs=hvT[:], start=True, stop=True)
                kt = p1.tile([128, N], F32, tag="kt")
                nc.vector.tensor_copy(out=kt[:], in_=kt_ps[:])
                for c in range(3):
                    a_ps = psum.tile([128, N], F32, tag="aps")
                    nc.tensor.matmul(out=a_ps[:], lhsT=qt[:, c * 128:(c + 1) * 128],
                                     rhs=kt[:], start=True, stop=True)
                    a_sb = p1.tile([128, N], F32, tag="asb")
                    nc.vector.tensor_copy(out=a_sb[:], in_=a_ps[:])
                    nc.sync.dma_start(out=a_dram[b * N + c * 128: b * N + (c + 1) * 128, :],
                                      in_=a_sb[:])
                b_ps = psum.tile([2, N], F32, tag="bps")
                nc.tensor.matmul(out=b_ps[:], lhsT=qh2_sb[:, 2 * b:2 * b + 2],
                                 rhs=kt[:], start=True, stop=True)
                b_sb = p1.tile([2, N], F32, tag="bsb")
                nc.vector.tensor_copy(out=b_sb[:], in_=b_ps[:])
                nc.sync.dma_start(out=bases_dram[b], in_=b_sb[:])

            base_sb = pers.tile([BL, N], F32, tag="base")
            nc.sync.dma_start(out=base_sb[:], in_=bases_dram[:, 0, :])
            x0_sb = pers.tile([BL, N], F32, tag="x0")
            nc.sync.dma_start(out=x0_sb[:], in_=bases_dram[:, 1, :])

            # ---------------- phase 2: sequential decode ----------------
            bf = pers.tile([BL, N], F32, tag="bf")            # base + G_first + visited*(-1e9)
            pi_sb = pers.tile([BL, N], I32, tag="pi")
            nc.vector.memset(pi_sb[:], 0)
            lnparts = pers.tile([BL, N], F32, tag="lnp")      # ln(sum exp) per step
            tanhm = pers.tile([BL, N], F32, tag="thm")        # tanh(max) per step
            iota_u32 = pers.tile([BL, N], U32, tag="iotau")
            nc.gpsimd.iota(iota_u32[:], pattern=[[1, N]], base=0, channel_multiplier=0)
            iota_row = pers.tile([BL, N], F32, tag="iota")
            nc.vector.tensor_copy(out=iota_row[:], in_=iota_u32[:])
            rowbase = pers.tile([BL, 1], U32, tag="rowb")
            nc.gpsimd.iota(rowbase[:], pattern=[[0, 1]], base=0, channel_multiplier=N)
            ones8 = pers.tile([BL, 8], F32, tag="ones8")
            nc.vector.memset(ones8[:], 1.0)
            neg1e6 = pers.tile([BL, 1], F32, tag="neg1e6")
            nc.vector.memset(neg1e6[:], NEG_MED)
            negone = pers.tile([BL, 1], F32, tag="negone")
            nc.vector.memset(negone[:], -1.0)
            e10 = pers.tile([BL, 1], F32, tag="e10")          # device's exp(-10)
            nc.scalar.activation(out=e10[:], in_=negone[:], func=Act.Exp, scale=EXP_NEG10_SCALE)

            def select_and_log(xm_ap, m1_ap, t):
                """Selection + logp bookkeeping for step t; returns aidx tile.

                m1_ap: [BL,1] max over free dim of xm_ap (from fused reduce)."""
                thr = lp.tile([BL, 1], F32, tag="thr")
                nc.vector.tensor_scalar(out=thr[:], in0=m1_ap, scalar1=L_SAT,
                                        scalar2=None, op0=Alu.min)
                negsat = lp.tile([BL, 1], U8, tag="negsat")
                nc.vector.tensor_scalar(out=negsat[:], in0=m1_ap, scalar1=-L_SAT,
                                        scalar2=None, op0=Alu.is_le)
                nc.vector.copy_predicated(out=thr[:], mask=negsat[:], data=neg1e6[:])
                y = lp.tile([BL, N], F32, tag="y")
                nc.vector.tensor_scalar(out=y[:], in0=xm_ap, scalar1=thr[:],
                                        scalar2=None, op0=Alu.is_ge)
                aidx = lp.tile([BL, 8], U32, tag="aidx")
                nc.vector.max_index(out=aidx[:], in_max=ones8[:], in_values=y[:])
                nc.vector.tensor_copy(out=pi_sb[:, t:t + 1], in_=aidx[:, 0:1])
                # ---- logp path (ACT engine, off critical path) ----
                th = lp.tile([BL, N], F32, tag="th")
                nc.scalar.activation(out=th[:], in_=xm_ap, func=Act.Tanh)
                ex = lp.tile([BL, N], F32, tag="ex")
                ssum = lp.tile([BL, 1], F32, tag="ssum")
                nc.scalar.activation(out=ex[:], in_=th[:], func=Act.Exp, scale=10.0,
                                     accum_out=ssum[:])
                if t > 0:
                    corr = lp.tile([BL, 1], F32, tag="corr")
                    nc.vector.tensor_scalar(out=corr[:], in0=e10[:], scalar1=float(t),
                                            scalar2=None, op0=Alu.mult)
                    ssc = lp.tile([BL, 1], F32, tag="ssc")
                    nc.vector.tensor_tensor(out=ssc[:], in0=ssum[:], in1=corr[:],
                                            op=Alu.subtract)
                else:
                    ssc = ssum
                nc.scalar.activation(out=lnparts[:, t:t + 1], in_=ssc[:], func=Act.Ln)
                nc.scalar.activation(out=tanhm[:, t:t + 1], in_=m1_ap, func=Act.Tanh)
                return aidx

            def mask_chosen(aidx):
                """bf[b, a_b] += -1e9"""
                af = lp.tile([BL, 1], F32, tag="af")
                nc.vector.tensor_copy(out=af[:], in_=aidx[:, 0:1])
                eqf = lp.tile([BL, N], F32, tag="eqf")
                nc.vector.tensor_scalar(out=eqf[:], in0=iota_row[:], scalar1=af[:, 0:1],
                                        scalar2=NEG_BIG, op0=Alu.is_equal, op1=Alu.mult)
                nc.vector.tensor_tensor(out=bf[:], in0=bf[:], in1=eqf[:], op=Alu.add)

            def gather_idx(aidx):
                idx = lp.tile([BL, 1], U32, tag="idx")
                nc.vector.tensor_tensor(out=idx[:], in0=aidx[:, 0:1], in1=rowbase[:],
                                        op=Alu.add)
                return idx

            # step 0 on x0
            m1_0 = lp.tile([BL, 1], F32, tag="m1")
            nc.vector.tensor_reduce(out=m1_0[:], in_=x0_sb[:], axis=AX_X, op=Alu.max)
            aidx = select_and_log(x0_sb[:], m1_0[:], 0)
            a0_idx = gather_idx(aidx)
            a0_aidx = aidx

            for t in range(1, n_steps):
                g = lp.tile([BL, N], F32, tag="g")
                nc.gpsimd.indirect_dma_start(
                    out=g[:], out_offset=None,
                    in_=a_dram[:],
                    in_offset=IndirectOffsetOnAxis(ap=(a0_idx if t == 1 else idx)[:, :1], axis=0),
                )
                xm = lp.tile([BL, N], F32, tag="xm")
                m1 = lp.tile([BL, 1], F32, tag="m1")
                if t == 1:
                    nc.vector.tensor_tensor(out=bf[:], in0=base_sb[:], in1=g[:], op=Alu.add)
                    mask_chosen(a0_aidx)
                nc.vector.tensor_tensor(out=xm[:], in0=bf[:], in1=g[:], op=Alu.add)
                nc.vector.tensor_reduce(out=m1[:], in_=xm[:], axis=AX_X, op=Alu.max)
                aidx = select_and_log(xm[:], m1[:], t)
                if t < n_steps - 1:
                    idx = gather_idx(aidx)
                    mask_chosen(aidx)

            # ---------------- finalize ----------------
            s1 = lp.tile([BL, 1], F32, tag="s1")
            nc.vector.reduce_sum(out=s1[:], in_=tanhm[:, 0:n_steps], axis=AX_X)
            s2 = lp.tile([BL, 1], F32, tag="s2")
            nc.vector.reduce_sum(out=s2[:], in_=lnparts[:, 0:n_steps], axis=AX_X)
            lp_t = lp.tile([BL, 1], F32, tag="lpt")
            nc.vector.tensor_scalar(out=lp_t[:], in0=s1[:], scalar1=10.0,
                                    scalar2=None, op0=Alu.mult)
            nc.vector.tensor_tensor(out=lp_t[:], in0=lp_t[:], in1=s2[:], op=Alu.subtract)
            nc.sync.dma_start(out=logp_out[:], in_=lp_t[:])
            nc.sync.dma_start(out=pi_out[:], in_=pi_sb[:, 0:N])

    if compile:
        nc.compile()
    return nc


def _host_prep(hvec, hbar, qv_p, kv_p, vec_1, vec_f):
    """Host-side prep: scale fold + context projections + per-core input maps."""
    hvec = np.asarray(hvec, dtype=np.float32)
    hbar = np.asarray(hbar, dtype=np.float32)
    qf = np.asarray(qv_p, dtype=np.float32).reshape(D, D)
    kf = np.asarray(kv_p, dtype=np.float32).reshape(D, D)
    vec_1 = np.asarray(vec_1, dtype=np.float32)
    vec_f = np.asarray(vec_f, dtype=np.float32)

    qf_sc = (np.float32(0.25) * qf).astype(np.float32)
    qhbar_sc = (hbar @ qf_sc).astype(np.float32)                     # [B,128]
    ctx0 = ((hbar + vec_1[None, :]).astype(np.float32) + vec_f[None, :]).astype(np.float32)
    q0_sc = (ctx0 @ qf_sc).astype(np.float32)                        # [B,128]

    in_maps = []
    for c in range(N_CORES):
        sl = slice(c * BL, (c + 1) * BL)
        qh2 = np.empty((D, 2 * BL), np.float32)
        qh2[:, 0::2] = qhbar_sc[sl].T
        qh2[:, 1::2] = q0_sc[sl].T
        in_maps.append({
            "hvec": np.ascontiguousarray(hvec[sl]),
            "qf": qf_sc,
            "kf": np.ascontiguousarray(kf),
            "qh2": qh2,
        })
    return in_maps


def kernel(hvec, hbar, qv_p, kv_p, vec_1, vec_f):
    in_maps = _host_prep(hvec, hbar, qv_p, kv_p, vec_1, vec_f)
    nc = build_nc()
    res = run_bass_kernel_spmd(nc, in_maps, list(range(N_CORES)))
    pi = np.concatenate([np.asarray(r["pi"]) for r in res.results], axis=0)   # [256, 384]
    logp = np.concatenate([np.asarray(r["logp"]).reshape(-1) for r in res.results])  # [256]
    return np.ascontiguousarray(pi.T.astype(np.int32)), logp.astype(np.float32)
